# revision 1
# baseline (speedup 1.0000x reference)
"""Trainium2 Bass kernel for a 12-head causal attention block (B=4, T=2048, C=768).

Sharding: 8 cores = 4 batches x 2 head-groups (6 heads each). Each core computes
q/k/v projections for its head-group over its batch's full sequence, causal
flash-style attention, and a partial output projection (row-parallel Wp).
Host sums the two partial outputs per batch. No cross-core collectives.

All matmul operands are fp16 (fp32 PSUM accumulation); measured end-to-end
relative error ~7e-4 vs the fp32 reference. Layouts are channel-major so no
on-chip transposes are needed:
  xT   [768, 2048]  x[b].T                        (fp16)
  wq/wk/wv [768, 384]  W[g*384:(g+1)*384, :].T    (fp16, lhsT layout)
  wp   [384, 768]  Wp[:, g*384:(g+1)*384].T       (fp16, lhsT layout)
  masks [4, 128, 512] causal mask tiles, ones [128, 64]   (fp16)
  out yT [768, 2048] fp32 partial = (attn_out_group @ Wp_group.T).T
"""

import numpy as np

T = 2048
C = 768
G = 384          # channels per head-group (6 heads x 64)
DH = 64
NK = C // 128    # 6 k-tiles over c_in
TBLK = 512
NTB = T // TBLK  # 4 t-blocks
NST = T // 128   # 16 s-tiles
N_CORES = 8

_CACHE = {}


def _emit(tc, yT, xT, wq, wk, wv, wp, masks, ones, dbg=None):
    import concourse.mybir as mybir

    nc = tc.nc
    DT = mybir.dt.float32
    H = mybir.dt.float16
    Exp = mybir.ActivationFunctionType.Exp
    mm = nc.tensor.matmul

    with (
        tc.tile_pool(name="pc", bufs=1) as pc,        # persistent sbuf
        tc.tile_pool(name="px", bufs=2) as px,        # x chunks
        tc.tile_pool(name="pe", bufs=6) as pe,        # exp tiles
        tc.tile_pool(name="pr", bufs=3) as pr,        # recip + y-out staging
        tc.tile_pool(name="pao", bufs=2) as pao,      # attn-out per t-block
        tc.tile_pool(name="psP", bufs=2, space="PSUM") as psP,    # scores
        tc.tile_pool(name="psQ", bufs=2, space="PSUM") as psQ,    # projections
        tc.tile_pool(name="pso", bufs=1, space="PSUM") as pso,    # attn accum
        tc.tile_pool(name="psd", bufs=1, space="PSUM") as psd,    # denom accum
    ):
        # ---- persistent tensors ----
        wq_sb = pc.tile([128, NK * G], H, tag="wq")
        wk_sb = pc.tile([128, NK * G], H, tag="wk")
        wv_sb = pc.tile([128, NK * G], H, tag="wv")
        nc.sync.dma_start(out=wq_sb.rearrange("p (k c) -> p k c", k=NK), in_=wq.rearrange("(k p) c -> p k c", p=128))
        nc.sync.dma_start(out=wk_sb.rearrange("p (k c) -> p k c", k=NK), in_=wk.rearrange("(k p) c -> p k c", p=128))
        nc.sync.dma_start(out=wv_sb.rearrange("p (k c) -> p k c", k=NK), in_=wv.rearrange("(k p) c -> p k c", p=128))
        wp_sb = pc.tile([128, 3 * C], H, tag="wp")
        masks_sb = pc.tile([128, 4, TBLK], H, tag="masks")
        ones_sb = pc.tile([128, 64], H, tag="ones")

        def late_loads():  # overlap with the first projection block
            nc.sync.dma_start(out=ones_sb[:], in_=ones)
            nc.sync.dma_start(out=masks_sb[:, :, :], in_=masks.rearrange("o p t -> p o t"))
            nc.sync.dma_start(out=wp_sb.rearrange("p (k c) -> p k c", k=3), in_=wp.rearrange("(k p) c -> p k c", p=128))

        # qT/kT: [128, 3*2048]; channel c of group -> partition c%128, block c//128.
        # head h (0..5): partitions (h%2)*64..+64 of block h//2.
        qT_sb = pc.tile([128, 3 * T], H, tag="qT")
        kT_sb = pc.tile([128, 3 * T], H, tag="kT")
        # v token-major: [128, 16*384]; col (st*6+h)*64 + m = v[st*128 + p, h*64 + m]
        v_sb = pc.tile([128, NST * G], H, tag="v")

        xT_r = xT.rearrange("(k p) t -> p k t", p=128)

        def load_chunks(tb):
            xt = px.tile([128, NK, TBLK], H, tag="xc")
            nc.sync.dma_start(out=xt[:], in_=xT_r[:, :, tb * TBLK:(tb + 1) * TBLK])
            return [xt[:, k, :] for k in range(NK)]

        def ph1_groups(tb, xc):
            # closures: one projection matmul group each (q/k x 3, v x 4)
            gs = []
            for w_sb, out_sb in ((wq_sb, qT_sb), (wk_sb, kT_sb)):
                for mo in range(3):
                    def g(w_sb=w_sb, out_sb=out_sb, mo=mo):
                        ps = psQ.tile([128, TBLK], DT, tag="pq")
                        for k in range(NK):
                            mm(ps[:, 0:TBLK],
                               lhsT=w_sb[:, k * G + mo * 128: k * G + (mo + 1) * 128],
                               rhs=xc[k], start=(k == 0), stop=(k == NK - 1))
                        nc.vector.tensor_copy(
                            out=out_sb[:, mo * T + tb * TBLK: mo * T + (tb + 1) * TBLK],
                            in_=ps[:, 0:TBLK])
                    gs.append(g)
            for sl in range(4):
                def g(sl=sl):
                    st = 4 * tb + sl
                    ps = psQ.tile([128, TBLK], DT, tag="pq")
                    for k in range(NK):
                        mm(ps[:, 0:G], lhsT=xc[k][:, sl * 128:(sl + 1) * 128],
                           rhs=wv_sb[:, k * G:(k + 1) * G], start=(k == 0), stop=(k == NK - 1))
                    nc.vector.tensor_copy(out=v_sb[:, st * G:(st + 1) * G], in_=ps[:, 0:G])
                gs.append(g)
            return gs

        def ph3_groups(tb, ao):
            gs = []
            for mo in range(6):
                def g(mo=mo):
                    py = psQ.tile([128, TBLK], DT, tag="pq")
                    for kk in range(3):
                        mm(py[:, 0:TBLK],
                           lhsT=wp_sb[:, kk * C + mo * 128: kk * C + (mo + 1) * 128],
                           rhs=ao[:, kk * TBLK:(kk + 1) * TBLK], start=(kk == 0), stop=(kk == 2))
                    yo = pr.tile([128, TBLK], DT, tag="yo")
                    nc.vector.tensor_copy(out=yo[:], in_=py[:, 0:TBLK])
                    nc.sync.dma_start(
                        out=yT[mo * 128:(mo + 1) * 128, tb * TBLK:(tb + 1) * TBLK], in_=yo[:])
                gs.append(g)
            return gs

        # serial head: projections for tb=0 (DMA-paced startup)
        xc_cur = load_chunks(0)
        head = ph1_groups(0, xc_cur)
        for g in head[:2]:
            g()
        late_loads()
        for g in head[2:]:
            g()

        queue = []       # projection groups to interleave into phase 2
        ph3_pending = []
        for tb in range(NTB):
            if tb < NTB - 1:
                xc_next = load_chunks(tb + 1)
                queue = ph3_pending + ph1_groups(tb + 1, xc_next)
            else:
                queue = list(ph3_pending)
            total_iters = 3 * 4 * (tb + 1)
            emitted = [0]

            def pop_queue(it, queue=None, emitted=None, total_iters=None):
                pass

            def make_pop(queue, emitted, total_iters):
                def pop_queue(it):
                    want = min(len(queue), (it + 1) * len(queue) // total_iters + 1)
                    while emitted[0] < want:
                        queue[emitted[0]]()
                        emitted[0] += 1
                return pop_queue
            pop_queue = make_pop(queue, emitted, total_iters)

            # ---- phase 2: attention for this t-block, head pairs j ----
            ao = pao.tile([128, 3 * TBLK], H, tag="ao")
            n_st = 4 * (tb + 1)

            norm_q = []

            def queue_norm(tAo_, tDen_, j_):
                rd = pr.tile([128, TBLK], DT, tag="rd")
                for q_ in range(4):
                    def g(q_=q_, rd=rd, tDen_=tDen_):
                        nc.vector.reciprocal(out=rd[:, q_ * 128:(q_ + 1) * 128],
                                             in_=tDen_[:, q_ * 128:(q_ + 1) * 128])
                    norm_q.append(g)
                for q_ in range(2):
                    def g(q_=q_, rd=rd, tAo_=tAo_, j_=j_):
                        nc.vector.tensor_mul(
                            ao[:, j_ * TBLK + q_ * 256: j_ * TBLK + (q_ + 1) * 256],
                            tAo_[:, q_ * 256:(q_ + 1) * 256], rd[:, q_ * 256:(q_ + 1) * 256])
                    norm_q.append(g)

            def pv_group(ent):
                j_, po_, pd_, st, e01, first, last = ent
                e0 = e01[:, 0:TBLK]
                e1 = e01[:, TBLK:2 * TBLK]
                s0 = (st * 6 + 2 * j_) * DH
                s1 = (st * 6 + 2 * j_ + 1) * DH
                mm(po_[0:64, :], lhsT=v_sb[:, s0:s0 + DH], rhs=e0,
                   start=first, stop=last, skip_group_check=True)
                mm(po_[64:128, :], lhsT=v_sb[:, s1:s1 + DH], rhs=e1,
                   start=first, stop=last, skip_group_check=True)
                mm(pd_[0:64, :], lhsT=ones_sb[:], rhs=e0,
                   start=first, stop=last, skip_group_check=True)
                mm(pd_[64:128, :], lhsT=ones_sb[:], rhs=e1,
                   start=first, stop=last, skip_group_check=True)
                if last:
                    # free the PSUM accumulators; normalize later from SBUF
                    tAo = pr.tile([128, TBLK], DT, tag="tAo")
                    tDen = pr.tile([128, TBLK], DT, tag="tDen")
                    nc.vector.tensor_copy(out=tAo[:], in_=po_[:])
                    nc.vector.tensor_copy(out=tDen[:], in_=pd_[:])
                    queue_norm(tAo, tDen, j_)

            pipe = []  # software pipeline: PV(st-2) is issued after scores(st)
            for j in range(3):
                po = pso.tile([128, TBLK], DT, tag="po")
                pd = psd.tile([128, TBLK], DT, tag="pd")
                qs = qT_sb[:, j * T + tb * TBLK: j * T + (tb + 1) * TBLK]
                for st in range(n_st):
                    ks = kT_sb[:, j * T + st * 128: j * T + st * 128 + 128]
                    ps = psP.tile([128, 2 * TBLK], DT, tag="pp")
                    mm(ps[:, 0:TBLK], lhsT=ks[0:64, :], rhs=qs[0:64, :], start=True, stop=True)
                    mm(ps[:, TBLK:2 * TBLK], lhsT=ks[64:128, :], rhs=qs[64:128, :], start=True, stop=True)
                    e01 = pe.tile([128, 2 * TBLK], H, tag="e01")
                    if st > 4 * tb:
                        # diagonal block: cols < 128*o are fully masked; zero them
                        # (gpsimd) and exp only the remainder
                        o = st - 4 * tb
                        nc.gpsimd.memset(
                            e01.rearrange("p (a b) -> p a b", a=2)[:, :, 0:128 * o], 0.0)
                        nc.scalar.activation(
                            out=e01.rearrange("p (a b) -> p a b", a=2)[:, :, 128 * o:],
                            in_=ps.rearrange("p (a b) -> p a b", a=2)[:, :, 128 * o:],
                            func=Exp, scale=float(DH) ** -0.5)
                    else:
                        nc.scalar.activation(out=e01[:], in_=ps[:], func=Exp,
                                             scale=float(DH) ** -0.5)
                    if st >= 4 * tb:  # triangular boundary block: causal mask
                        o = st - 4 * tb
                        lo, hi = 128 * o, 128 * (o + 1)
                        nc.vector.tensor_mul(e01[:, lo:hi], e01[:, lo:hi],
                                             masks_sb[:, o, lo:hi])
                        nc.vector.tensor_mul(e01[:, TBLK + lo:TBLK + hi],
                                             e01[:, TBLK + lo:TBLK + hi],
                                             masks_sb[:, o, lo:hi])
                    pipe.append((j, po, pd, st, e01, st == 0, st == n_st - 1))
                    if len(pipe) > 2:
                        pv_group(pipe.pop(0))
                    if norm_q:
                        norm_q.pop(0)()  # previous j's normalize, off the critical path
                    pop_queue(j * n_st + st)
                for ent in pipe:   # flush within j (po/pd are single-buffered)
                    pv_group(ent)
                pipe = []
            while norm_q:
                norm_q.pop(0)()
            while emitted[0] < len(queue):
                queue[emitted[0]]()
                emitted[0] += 1
            ph3_pending = ph3_groups(tb, ao)
        for g in ph3_pending:  # tail: projection of the last t-block
            g()


def build_program():
    if "nc" in _CACHE:
        return _CACHE["nc"]
    import concourse.bacc as bacc
    import concourse.tile as tile
    import concourse.mybir as mybir

    nc = bacc.Bacc("TRN2", target_bir_lowering=False, debug=False)
    DT = mybir.dt.float32
    H = mybir.dt.float16
    xT_d = nc.dram_tensor("xT", [C, T], H, kind="ExternalInput")
    wq_d = nc.dram_tensor("wq", [C, G], H, kind="ExternalInput")
    wk_d = nc.dram_tensor("wk", [C, G], H, kind="ExternalInput")
    wv_d = nc.dram_tensor("wv", [C, G], H, kind="ExternalInput")
    wp_d = nc.dram_tensor("wp", [G, C], H, kind="ExternalInput")
    mk_d = nc.dram_tensor("masks", [4, 128, TBLK], H, kind="ExternalInput")
    on_d = nc.dram_tensor("ones", [128, 64], H, kind="ExternalInput")
    yT_d = nc.dram_tensor("yT", [C, T], DT, kind="ExternalOutput")

    with tile.TileContext(nc) as tc:
        _emit(tc, yT_d.ap(), xT_d.ap(), wq_d.ap(), wk_d.ap(), wv_d.ap(),
              wp_d.ap(), mk_d.ap(), on_d.ap())
    nc.compile()
    _CACHE["nc"] = nc
    return nc


def make_masks():
    s = np.arange(128)[:, None]
    t = np.arange(TBLK)[None, :]
    return np.stack([(t >= 128 * o + s) for o in range(4)]).astype(np.float16)


def shard_inputs(x, Wq, Wk, Wv, Wp):
    """Full inputs -> list of 8 per-core input dicts (fp16 operands)."""
    x = np.asarray(x, dtype=np.float32)
    Wq, Wk, Wv, Wp = (np.asarray(w, dtype=np.float32) for w in (Wq, Wk, Wv, Wp))
    masks = make_masks()
    ones = np.ones((128, 64), dtype=np.float16)
    in_maps = []
    for c in range(N_CORES):
        b, g = divmod(c, 2)
        sl = slice(g * G, (g + 1) * G)
        in_maps.append({
            "xT": np.ascontiguousarray(x[b].T).astype(np.float16),
            "wq": np.ascontiguousarray(Wq[sl, :].T).astype(np.float16),
            "wk": np.ascontiguousarray(Wk[sl, :].T).astype(np.float16),
            "wv": np.ascontiguousarray(Wv[sl, :].T).astype(np.float16),
            "wp": np.ascontiguousarray(Wp[:, sl].T).astype(np.float16),
            "masks": masks,
            "ones": ones,
        })
    return in_maps


def combine_outputs(results):
    """Per-core {'yT': [768,2048]} partials -> full [4, 2048, 768] output."""
    out = np.empty((4, T, C), dtype=np.float32)
    for b in range(4):
        acc = results[2 * b]["yT"] + results[2 * b + 1]["yT"]
        out[b] = acc.T
    return out


def kernel(x, Wq, Wk, Wv, Wp, **run_kwargs):
    from concourse.bass_utils import run_bass_kernel_spmd

    nc = build_program()
    in_maps = shard_inputs(x, Wq, Wk, Wv, Wp)
    res = run_bass_kernel_spmd(nc, in_maps, core_ids=list(range(N_CORES)), **run_kwargs)
    out = combine_outputs(res.results)
    if run_kwargs:
        return out, res
    return out

